# revision 1
# baseline (speedup 1.0000x reference)
"""Trainium2 Bass kernel for a basic tanh RNN + output projection.

Reference computation (all fp32):
    s_t = tanh(x[:, :, t] @ Wx + s_{t-1} @ Wh + b)      t = 0..T-1, s_{-1} = 0
    out[:, t, :] = s_t @ Wout + bout

Shapes: x (64, 256, 1024), Wx (256, 1024), Wh (1024, 1024), b (1024,),
        Wout (1024, 512), bout (512,)  ->  out (64, 1024, 512)

Strategy (8 NeuronCores):
  The T=1024 recurrence is sequential, so every core runs the full-batch
  recurrence replicated (state kept transposed [H, B] on partitions), and
  only the output projection + output writes are sharded by batch.  Each
  core receives x with the batch axis rotated so its own 8 batch columns
  sit at positions 0..7; all cores run one identical program (SPMD).

  Schedule (per step, measured 2.780 ms total on hw, rel err 6.05e-3):

    [1 projection matmul (moving 256; drains the previous window at one
     matmul per step)] [Wx for m-blocks 0..5] [Wh m0 k0..5] [m1 k0..5]
    [m0 k6,k7] [m1 k6,k7] [tanh pair0] [m2] [m3] [tanh pair1] [Wx m6,m7]
    [m4] [m5] [tanh pair2] [m6] [m7] [tanh pair3]

  Rationale, from hw experiments (see iterate.py for the variant bench):
  - A free-running variant (recurrence dependency broken) measures
    1.02 ms: the PE does bf16 matmuls at ~0.19 ns/moving-row, >2x the
    published 78.6 TF/s, so the kernel is LATENCY-bound: the binding
    chain is psum-stop -> sem -> ScalarE tanh -> sem -> next step's Wh.
    The schedule gives the PE ~700 ns of state-independent work (proj +
    hoisted Wx) at each step head before the first s_{t-1}[k>=6] use.
  - PSUM is 8 banks, tiles are bank-granular, and start=True ZEROES THE
    WHOLE BANK, killing any other open accumulation there (verified on
    hw).  Two m-blocks therefore share one bank as a [128,2,64] tile
    that DVE pre-fills with the bias each step (tensor_copy), and every
    matmul accumulates with start=False (verified exact on hw).  Banks:
    pair01 x2 + pair23 x2 + pair45 x2 + pair67 x1 + proj x1 = 8.
  - ScalarE instructions carry a ~185 ns non-pipelineable access-latency
    cost, so 8 single-block tanhs/step (~2.2 us busy) saturate the Act
    engine and snowball the critical path (measured 3.06 ms); fusing
    each pair into one [128,2,64] tanh (bias already in psum) gives 4
    tanhs (~1.2 us busy, no backlog).  DVE has no tanh.
  - m6/m7's Wx is issued mid-step because their single-buffered bank is
    only freed ~650 ns into the step by the previous step's pair67 tanh.
  - Splitting the batch into two interleaved 32-wide streams (to hide
    the tanh latency entirely) measures 5.4 ms: moving-32 matmuls can no
    longer hide the stationary weight load, so never shrink moving below
    64.  fp8 was rejected on accuracy: the recurrence amplifies per-step
    quantization ~10x (e4m3 weights alone 6.6e-2 rel err vs the 2e-2
    gate; bf16 6e-3).
"""

import numpy as np
import ml_dtypes

import concourse.bass as bass
from concourse import bacc
import concourse.mybir as mybir
import concourse.tile as tile
from concourse.bass_utils import run_bass_kernel_spmd

B, F, T = 64, 256, 1024
H, O = 1024, 512
NCORES = 8
MB = B // NCORES  # own-batch columns per core (projection shard)
P = 128
KH, KF, MH, OBK = H // P, F // P, H // P, O // P  # 8, 2, 8, 4
NPAIR = MH // 2

BF16 = mybir.dt.bfloat16
F32 = mybir.dt.float32
np_bf16 = ml_dtypes.bfloat16


def build_program(
    t_steps: int = T,
    w_steps: int = 64,
    zbufs: int = 4,      # unused; kept for test.py compatibility
    proj_every: int = 2,  # unused; kept for compatibility
    reps: int = 1,
    parity: bool = True,  # layout is always parity-split in this version
    defer_k7: int = 1,    # 0: none, 1: defer k6/k7 of m0/m1, 2: k-major m0..m3
    sbufs: int = 2,
    g_dma: int = 1,       # unused; kept for compatibility
    free_run: bool = False,  # TIMING DIAGNOSTIC: break the recurrence dep
) -> bass.Bass:
    assert t_steps % w_steps == 0
    assert parity, "this version always uses the parity stage layout"
    nw = t_steps // w_steps
    pw = w_steps * MB        # projection moving size per window (512)
    half = pw // 2
    hw = w_steps // 2
    # proj drain: one matmul per step; pieces per ob chunk = 2*MH = 16
    ppc = 2 * MH
    assert w_steps == OBK * ppc, "proj drain schedule needs w_steps == 64"

    nc = bacc.Bacc()

    xt_d = nc.declare_dram_parameter("xt", [t_steps, F, B], BF16, isOutput=False)
    wh_d = nc.declare_dram_parameter("wh", [H, H], BF16, isOutput=False)
    wx_d = nc.declare_dram_parameter("wx", [F, H], BF16, isOutput=False)
    wo_d = nc.declare_dram_parameter("wout", [H, O], BF16, isOutput=False)
    # bias pre-broadcast along batch: bbc[p, m, b] = bias[m*128 + p]
    bbc_d = nc.declare_dram_parameter("bvecbc", [P, KH, B], F32, isOutput=False)
    bo_d = nc.declare_dram_parameter("boutvec", [O], F32, isOutput=False)
    out_d = nc.declare_dram_parameter("out", [nw, OBK, P, pw], F32, isOutput=True)

    with tile.TileContext(nc) as tc:
        with (
            tc.tile_pool(name="const", bufs=1) as cpool,
            tc.tile_pool(name="stage", bufs=sbufs) as spool,
            tc.tile_pool(name="xin", bufs=6) as xpool,
            tc.tile_pool(name="outsb", bufs=4) as opool,
            tc.tile_pool(name="psz", bufs=1, space="PSUM") as zpool,
            tc.tile_pool(name="psp", bufs=1, space="PSUM") as ppool,
        ):
            # --- resident weights ---------------------------------------
            wh_sb = cpool.tile([P, KH, H], BF16, tag="wh")
            nc.sync.dma_start(wh_sb[:], wh_d.rearrange("(kb p) c -> p kb c", p=P))
            wx_sb = cpool.tile([P, KF, H], BF16, tag="wx")
            nc.sync.dma_start(wx_sb[:], wx_d.rearrange("(kb p) c -> p kb c", p=P))
            wo_sb = cpool.tile([P, MH, O], BF16, tag="wo")
            nc.sync.dma_start(wo_sb[:], wo_d.rearrange("(kb p) c -> p kb c", p=P))
            bbc_sb = cpool.tile([P, KH, B], F32, tag="bbc")
            nc.sync.dma_start(bbc_sb[:], bbc_d[:, :, :])
            bo_sb = cpool.tile([P, OBK], F32, tag="bo")
            nc.sync.dma_start(bo_sb[:], bo_d.rearrange("(m p) -> p m", p=P))
            dummy_sb = None
            if free_run:
                dummy_sb = cpool.tile([P, B], BF16, tag="dummy")
                nc.vector.memset(dummy_sb, 0.0)

            def emit_whole_kernel():
                stage_prev = None
                stage_cur = None
                pending_proj = []  # (window_idx, stage_tiles)
                proj_pp = [None]   # current ob-chunk psum tile (pre-zeroed)

                def new_stage():
                    # stage[j][par]: [P, 2, hw, B]; m-blocks 2j, 2j+1 fused
                    return [
                        [
                            spool.tile(
                                [P, 2, hw, B], BF16,
                                tag=f"stage{j}p{par}", name=f"stage{j}p{par}",
                            )
                            for par in range(2)
                        ]
                        for j in range(NPAIR)
                    ]

                def state_slice(stage, k, t_local):
                    # [P, B] state block k at local step t_local
                    return stage[k // 2][t_local % 2][:, k % 2, t_local // 2, :]

                def prep_pp():
                    proj_pp[0] = ppool.tile([P, pw], F32, tag="pproj", name="pproj")
                    nc.vector.memset(proj_pp[0], 0.0)

                def emit_proj_piece(w_idx, stiles, ob, j):
                    # piece j of ob's [P, pw] chunk: par = j%2, m = j//2
                    par, m = j % 2, j // 2
                    pp = proj_pp[0]
                    nc.tensor.matmul(
                        pp[:, par * half : (par + 1) * half],
                        wo_sb[:, m, ob * P : (ob + 1) * P],
                        stiles[m // 2][par][:, m % 2, :, 0:MB],
                        start=False,
                        stop=(m == MH - 1),
                        skip_group_check=True,
                    )
                    if j == ppc - 1:
                        osb = opool.tile([P, pw], F32, tag="osb", name="osb")
                        nc.vector.tensor_scalar_add(osb, pp, bo_sb[:, ob : ob + 1])
                        nc.sync.dma_start(out_d[w_idx, ob], osb)
                        prep_pp()  # zero the bank for the next chunk

                def pz_slot(pz, m):
                    return pz[m // 2][:, m % 2, :]

                def emit_wx(pz, xt_sb, m):
                    for kf in range(KF):
                        nc.tensor.matmul(
                            pz_slot(pz, m),
                            wx_sb[:, kf, m * P : (m + 1) * P],
                            xt_sb[:, kf, :],
                            start=False,
                            stop=False,
                            skip_group_check=True,
                        )

                def emit_wh(pz, prev, ptl, m, ks, stop_at):
                    for k in ks:
                        nc.tensor.matmul(
                            pz_slot(pz, m),
                            wh_sb[:, k, m * P : (m + 1) * P],
                            dummy_sb if free_run else state_slice(prev, k, ptl),
                            start=False,
                            stop=(k == stop_at),
                            skip_group_check=True,
                        )

                def emit_tanh(pz, j, tl):
                    # fused: both m-blocks of pair j in one Act instruction
                    nc.scalar.activation(
                        stage_cur[j][tl % 2][:, :, tl // 2, :],
                        pz[j],
                        mybir.ActivationFunctionType.Tanh,
                    )

                for t in range(t_steps):
                    tl = t % w_steps
                    if tl == 0:
                        stage_prev = stage_cur
                        stage_cur = new_stage()

                    xt_sb = xpool.tile([P, KF, B], BF16, tag="xt", name="xt")
                    nc.sync.dma_start(
                        xt_sb[:], xt_d[t].rearrange("(kb p) b -> p kb b", p=P)
                    )

                    # psum pair tiles: pair j holds m-blocks 2j and 2j+1.
                    # DVE preloads the bias each step; matmuls accumulate
                    # with start=False; one fused tanh per pair.
                    pz = [
                        zpool.tile(
                            [P, 2, B], F32,
                            tag=f"pair{j}", bufs=(2 if j < 3 else 1),
                            name=f"pair{j}",
                        )
                        for j in range(NPAIR)
                    ]
                    for j in range(NPAIR):
                        nc.vector.tensor_copy(
                            pz[j], bbc_sb[:, 2 * j : 2 * j + 2, :]
                        )

                    # --- projection filler: one matmul per step ----------
                    if pending_proj:
                        w_idx, stiles = pending_proj[0]
                        emit_proj_piece(w_idx, stiles, tl // ppc, tl % ppc)
                        if tl == w_steps - 1:
                            pending_proj.pop(0)

                    # --- hoisted Wx for m0..m5 ---------------------------
                    for m in range(6):
                        emit_wx(pz, xt_sb, m)

                    if t == 0:
                        for m in (6, 7):
                            emit_wx(pz, xt_sb, m)
                        for j in range(NPAIR):
                            emit_tanh(pz, j, tl)
                    else:
                        prev = stage_cur if tl > 0 else stage_prev
                        ptl = (t - 1) % w_steps
                        if defer_k7 == 2:
                            # k-major over m0..m3: first k6/k7 use lands
                            # ~1100ns into the step, far past tanh latency
                            for k in range(KH - 2):
                                for m in range(4):
                                    emit_wh(pz, prev, ptl, m, [k], stop_at=-1)
                            for m in (0, 1):
                                emit_wh(pz, prev, ptl, m,
                                        [KH - 2, KH - 1], stop_at=KH - 1)
                            emit_tanh(pz, 0, tl)
                            for m in (2, 3):
                                emit_wh(pz, prev, ptl, m,
                                        [KH - 2, KH - 1], stop_at=KH - 1)
                            emit_tanh(pz, 1, tl)
                        elif defer_k7:
                            # defer k6/k7 of m0/m1: pair3's fused tanh makes
                            # s[6] only as fresh as s[7] (~500ns into step t)
                            emit_wh(pz, prev, ptl, 0, range(KH - 2), stop_at=-1)
                            emit_wh(pz, prev, ptl, 1, range(KH - 2), stop_at=-1)
                            emit_wh(pz, prev, ptl, 0, [KH - 2, KH - 1], stop_at=KH - 1)
                            emit_wh(pz, prev, ptl, 1, [KH - 2, KH - 1], stop_at=KH - 1)
                            emit_tanh(pz, 0, tl)
                            emit_wh(pz, prev, ptl, 2, range(KH), stop_at=KH - 1)
                            emit_wh(pz, prev, ptl, 3, range(KH), stop_at=KH - 1)
                            emit_tanh(pz, 1, tl)
                        else:
                            emit_wh(pz, prev, ptl, 0, range(KH), stop_at=KH - 1)
                            emit_wh(pz, prev, ptl, 1, range(KH), stop_at=KH - 1)
                            emit_tanh(pz, 0, tl)
                            emit_wh(pz, prev, ptl, 2, range(KH), stop_at=KH - 1)
                            emit_wh(pz, prev, ptl, 3, range(KH), stop_at=KH - 1)
                            emit_tanh(pz, 1, tl)
                        # m6/m7 Wx mid-step: their bank frees ~650ns in
                        for m in (6, 7):
                            emit_wx(pz, xt_sb, m)
                        emit_wh(pz, prev, ptl, 4, range(KH), stop_at=KH - 1)
                        emit_wh(pz, prev, ptl, 5, range(KH), stop_at=KH - 1)
                        emit_tanh(pz, 2, tl)
                        emit_wh(pz, prev, ptl, 6, range(KH), stop_at=KH - 1)
                        emit_wh(pz, prev, ptl, 7, range(KH), stop_at=KH - 1)
                        emit_tanh(pz, 3, tl)

                    if tl == w_steps - 1:
                        pending_proj.append((t // w_steps, stage_cur))
                        if proj_pp[0] is None:
                            prep_pp()  # bootstrap before the first drain

                # tail: drain remaining windows
                for w_idx, stiles in pending_proj:
                    for ob in range(OBK):
                        for j in range(ppc):
                            emit_proj_piece(w_idx, stiles, ob, j)

            if reps > 1:
                with tc.For_i(0, reps, 1):
                    emit_whole_kernel()
            else:
                emit_whole_kernel()

    nc.compile()
    return nc


def _host_prep(x, Wx, Wh, b, Wout, bout, t_steps):
    """Build the 8 per-core input maps."""
    xt = np.ascontiguousarray(x[:, :, :t_steps].transpose(2, 1, 0)).astype(np_bf16)
    wh = Wh.astype(np_bf16)
    wx = Wx.astype(np_bf16)
    wo = Wout.astype(np_bf16)
    bbc = np.ascontiguousarray(
        np.broadcast_to(
            np.asarray(b, np.float32).reshape(KH, P).T[:, :, None], (P, KH, B)
        )
    )
    bo = np.ascontiguousarray(bout, dtype=np.float32)
    in_maps = []
    for c in range(NCORES):
        xt_c = np.ascontiguousarray(np.roll(xt, -MB * c, axis=2))
        in_maps.append(
            {
                "xt": xt_c,
                "wh": wh,
                "wx": wx,
                "wout": wo,
                "bvecbc": bbc,
                "boutvec": bo,
            }
        )
    return in_maps


def _assemble(results, t_steps, w_steps, parity=True):
    nw = t_steps // w_steps
    out = np.empty((B, t_steps, O), np.float32)
    for c in range(NCORES):
        arr = results[c]["out"].reshape(nw, OBK, P, 2, w_steps // 2, MB)
        # out[MB*c+j, w*W + tt*2 + par, ob*P + p] = arr[w, ob, p, par, tt, j]
        out[MB * c : MB * (c + 1)] = (
            arr.transpose(5, 0, 4, 3, 1, 2).reshape(MB, t_steps, O)
        )
    return out


def run(
    x, Wx, Wh, b, Wout, bout,
    t_steps=T, w_steps=64, zbufs=4, parity=True, trace=False,
):
    nc = build_program(t_steps, w_steps, zbufs=zbufs, parity=parity)
    in_maps = _host_prep(x, Wx, Wh, b, Wout, bout, t_steps)
    res = run_bass_kernel_spmd(nc, in_maps, list(range(NCORES)), trace=trace)
    out = _assemble(res.results, t_steps, w_steps, parity=parity)
    return out, res


def kernel(x, Wx, Wh, b, Wout, bout):
    out, _ = run(
        np.asarray(x, dtype=np.float32),
        np.asarray(Wx, dtype=np.float32),
        np.asarray(Wh, dtype=np.float32),
        np.asarray(b, dtype=np.float32),
        np.asarray(Wout, dtype=np.float32),
        np.asarray(bout, dtype=np.float32),
    )
    return out



# revision 7
# speedup vs baseline: 1.0210x; 1.0210x over previous
"""Trainium2 Bass kernel for a basic tanh RNN + output projection.

Reference computation (all fp32):
    s_t = tanh(x[:, :, t] @ Wx + s_{t-1} @ Wh + b)      t = 0..T-1, s_{-1} = 0
    out[:, t, :] = s_t @ Wout + bout

Shapes: x (64, 256, 1024), Wx (256, 1024), Wh (1024, 1024), b (1024,),
        Wout (1024, 512), bout (512,)  ->  out (64, 1024, 512)

Strategy (8 NeuronCores):
  The T=1024 recurrence is sequential, so every core runs the full-batch
  recurrence replicated (state kept transposed [H, B] on partitions), and
  only the output projection + output writes are sharded by batch.  Each
  core receives x with the batch axis rotated so its own 8 batch columns
  sit at positions 0..7; all cores run one identical program (SPMD).

  Schedule (per step, measured 2.780 ms total on hw, rel err 6.05e-3):

    [1 projection matmul (moving 256; drains the previous window at one
     matmul per step)] [Wx for m-blocks 0..5] [Wh m0 k0..5] [m1 k0..5]
    [m0 k6,k7] [m1 k6,k7] [tanh pair0] [m2] [m3] [tanh pair1] [Wx m6,m7]
    [m4] [m5] [tanh pair2] [m6] [m7] [tanh pair3]

  Rationale, from hw experiments (see iterate.py for the variant bench):
  - A free-running variant (recurrence dependency broken) measures
    1.02 ms: the PE does bf16 matmuls at ~0.19 ns/moving-row, >2x the
    published 78.6 TF/s, so the kernel is LATENCY-bound: the binding
    chain is psum-stop -> sem -> ScalarE tanh -> sem -> next step's Wh.
    The schedule gives the PE ~700 ns of state-independent work (proj +
    hoisted Wx) at each step head before the first s_{t-1}[k>=6] use.
  - PSUM is 8 banks, tiles are bank-granular, and start=True ZEROES THE
    WHOLE BANK, killing any other open accumulation there (verified on
    hw).  Two m-blocks therefore share one bank as a [128,2,64] tile
    that DVE pre-fills with the bias each step (tensor_copy), and every
    matmul accumulates with start=False (verified exact on hw).  Banks:
    pair01 x2 + pair23 x2 + pair45 x2 + pair67 x1 + proj x1 = 8.
  - ScalarE instructions carry a ~185 ns non-pipelineable access-latency
    cost, so 8 single-block tanhs/step (~2.2 us busy) saturate the Act
    engine and snowball the critical path (measured 3.06 ms); fusing
    each pair into one [128,2,64] tanh (bias already in psum) gives 4
    tanhs (~1.2 us busy, no backlog).  DVE has no tanh.
  - m6/m7's Wx is issued mid-step because their single-buffered bank is
    only freed ~650 ns into the step by the previous step's pair67 tanh.
  - Splitting the batch into two interleaved 32-wide streams (to hide
    the tanh latency entirely) measures 5.4 ms: moving-32 matmuls can no
    longer hide the stationary weight load, so never shrink moving below
    64.  fp8 was rejected on accuracy: the recurrence amplifies per-step
    quantization ~10x (e4m3 weights alone 6.6e-2 rel err vs the 2e-2
    gate; bf16 6e-3).
"""

import numpy as np
import ml_dtypes

import concourse.bass as bass
from concourse import bacc
import concourse.mybir as mybir
import concourse.tile as tile
from concourse.bass_utils import run_bass_kernel_spmd

B, F, T = 64, 256, 1024
H, O = 1024, 512
NCORES = 8
MB = B // NCORES  # own-batch columns per core (projection shard)
P = 128
KH, KF, MH, OBK = H // P, F // P, H // P, O // P  # 8, 2, 8, 4
NPAIR = MH // 2

BF16 = mybir.dt.bfloat16
F32 = mybir.dt.float32
np_bf16 = ml_dtypes.bfloat16


def build_program(
    t_steps: int = T,
    w_steps: int = 64,
    zbufs: int = 4,      # unused; kept for test.py compatibility
    proj_every: int = 2,  # unused; kept for compatibility
    reps: int = 1,
    parity: bool = True,  # layout is always parity-split in this version
    defer_k7: int = 1,    # 0: none, 1: defer k6/k7 of m0/m1, 2: k-major m0..m3
    sbufs: int = 2,
    g_dma: int = 1,       # unused; kept for compatibility
    free_run: bool = False,  # TIMING DIAGNOSTIC: break the recurrence dep
) -> bass.Bass:
    assert t_steps % w_steps == 0
    assert parity, "this version always uses the parity stage layout"
    nw = t_steps // w_steps
    pw = w_steps * MB        # projection moving size per window (512)
    half = pw // 2
    hw = w_steps // 2
    # proj drain: one matmul per step; pieces per ob chunk = 2*MH = 16
    ppc = 2 * MH
    assert w_steps == OBK * ppc, "proj drain schedule needs w_steps == 64"

    nc = bacc.Bacc()

    xt_d = nc.declare_dram_parameter("xt", [t_steps, F, B], BF16, isOutput=False)
    wh_d = nc.declare_dram_parameter("wh", [H, H], BF16, isOutput=False)
    wx_d = nc.declare_dram_parameter("wx", [F, H], BF16, isOutput=False)
    wo_d = nc.declare_dram_parameter("wout", [H, O], BF16, isOutput=False)
    # bias pre-broadcast along batch: bbc[p, m, b] = bias[m*128 + p]
    bbc_d = nc.declare_dram_parameter("bvecbc", [P, KH, B], F32, isOutput=False)
    bo_d = nc.declare_dram_parameter("boutvec", [O], F32, isOutput=False)
    out_d = nc.declare_dram_parameter("out", [nw, OBK, P, pw], F32, isOutput=True)

    with tile.TileContext(nc) as tc:
        with (
            tc.tile_pool(name="const", bufs=1) as cpool,
            tc.tile_pool(name="stage", bufs=sbufs) as spool,
            tc.tile_pool(name="xin", bufs=6) as xpool,
            tc.tile_pool(name="outsb", bufs=4) as opool,
            tc.tile_pool(name="psz", bufs=1, space="PSUM") as zpool,
            tc.tile_pool(name="psp", bufs=1, space="PSUM") as ppool,
        ):
            # --- resident weights ---------------------------------------
            wh_sb = cpool.tile([P, KH, H], BF16, tag="wh")
            nc.sync.dma_start(wh_sb[:], wh_d.rearrange("(kb p) c -> p kb c", p=P))
            wx_sb = cpool.tile([P, KF, H], BF16, tag="wx")
            nc.sync.dma_start(wx_sb[:], wx_d.rearrange("(kb p) c -> p kb c", p=P))
            wo_sb = cpool.tile([P, MH, O], BF16, tag="wo")
            nc.sync.dma_start(wo_sb[:], wo_d.rearrange("(kb p) c -> p kb c", p=P))
            bbc_sb = cpool.tile([P, KH, B], F32, tag="bbc")
            nc.sync.dma_start(bbc_sb[:], bbc_d[:, :, :])
            bo_sb = cpool.tile([P, OBK], F32, tag="bo")
            nc.sync.dma_start(bo_sb[:], bo_d.rearrange("(m p) -> p m", p=P))
            dummy_sb = None
            if free_run:
                dummy_sb = cpool.tile([P, B], BF16, tag="dummy")
                nc.vector.memset(dummy_sb, 0.0)

            def emit_whole_kernel():
                stage_prev = None
                stage_cur = None
                pending_proj = []  # (window_idx, stage_tiles)
                proj_pp = [None]   # current ob-chunk psum tile (pre-zeroed)

                def new_stage():
                    # stage[j][par]: [P, 2, hw, B]; m-blocks 2j, 2j+1 fused
                    return [
                        [
                            spool.tile(
                                [P, 2, hw, B], BF16,
                                tag=f"stage{j}p{par}", name=f"stage{j}p{par}",
                            )
                            for par in range(2)
                        ]
                        for j in range(NPAIR)
                    ]

                def state_slice(stage, k, t_local):
                    # [P, B] state block k at local step t_local
                    return stage[k // 2][t_local % 2][:, k % 2, t_local // 2, :]

                def prep_pp():
                    proj_pp[0] = ppool.tile([P, pw], F32, tag="pproj", name="pproj")
                    nc.vector.memset(proj_pp[0], 0.0)

                def emit_proj_piece(w_idx, stiles, ob, j):
                    # piece j of ob's [P, pw] chunk: par = j%2, m = j//2
                    par, m = j % 2, j // 2
                    pp = proj_pp[0]
                    nc.tensor.matmul(
                        pp[:, par * half : (par + 1) * half],
                        wo_sb[:, m, ob * P : (ob + 1) * P],
                        stiles[m // 2][par][:, m % 2, :, 0:MB],
                        start=False,
                        stop=(m == MH - 1),
                        skip_group_check=True,
                    )
                    if j == ppc - 1:
                        osb = opool.tile([P, pw], F32, tag="osb", name="osb")
                        nc.vector.tensor_scalar_add(osb, pp, bo_sb[:, ob : ob + 1])
                        nc.sync.dma_start(out_d[w_idx, ob], osb)
                        prep_pp()  # zero the bank for the next chunk

                def pz_slot(pz, m):
                    return pz[m // 2][:, m % 2, :]

                def emit_wx(pz, xt_sb, m):
                    for kf in range(KF):
                        nc.tensor.matmul(
                            pz_slot(pz, m),
                            wx_sb[:, kf, m * P : (m + 1) * P],
                            xt_sb[:, kf, :],
                            start=False,
                            stop=False,
                            skip_group_check=True,
                        )

                def emit_wh(pz, prev, ptl, m, ks, stop_at):
                    for k in ks:
                        nc.tensor.matmul(
                            pz_slot(pz, m),
                            wh_sb[:, k, m * P : (m + 1) * P],
                            dummy_sb if free_run else state_slice(prev, k, ptl),
                            start=False,
                            stop=(k == stop_at),
                            skip_group_check=True,
                        )

                def emit_tanh(pz, j, tl):
                    # fused: both m-blocks of pair j in one Act instruction
                    nc.scalar.activation(
                        stage_cur[j][tl % 2][:, :, tl // 2, :],
                        pz[j],
                        mybir.ActivationFunctionType.Tanh,
                    )

                for t in range(t_steps):
                    tl = t % w_steps
                    if tl == 0:
                        stage_prev = stage_cur
                        stage_cur = new_stage()

                    xt_sb = xpool.tile([P, KF, B], BF16, tag="xt", name="xt")
                    nc.sync.dma_start(
                        xt_sb[:], xt_d[t].rearrange("(kb p) b -> p kb b", p=P)
                    )

                    # psum pair tiles: pair j holds m-blocks 2j and 2j+1.
                    # DVE preloads the bias each step; matmuls accumulate
                    # with start=False; one fused tanh per pair.
                    pz = [
                        zpool.tile(
                            [P, 2, B], F32,
                            tag=f"pair{j}", bufs=(2 if j < 3 else 1),
                            name=f"pair{j}",
                        )
                        for j in range(NPAIR)
                    ]
                    for j in range(NPAIR):
                        nc.vector.tensor_copy(
                            pz[j], bbc_sb[:, 2 * j : 2 * j + 2, :]
                        )

                    # --- projection filler: one matmul per step ----------
                    if pending_proj:
                        w_idx, stiles = pending_proj[0]
                        emit_proj_piece(w_idx, stiles, tl // ppc, tl % ppc)
                        if tl == w_steps - 1:
                            pending_proj.pop(0)

                    # --- hoisted Wx for m0..m5 ---------------------------
                    for m in range(6):
                        emit_wx(pz, xt_sb, m)

                    if t == 0:
                        for m in (6, 7):
                            emit_wx(pz, xt_sb, m)
                        for j in range(NPAIR):
                            emit_tanh(pz, j, tl)
                    else:
                        prev = stage_cur if tl > 0 else stage_prev
                        ptl = (t - 1) % w_steps
                        if defer_k7 == 2:
                            # k-major over m0..m3: first k6/k7 use lands
                            # ~1100ns into the step, far past tanh latency
                            for k in range(KH - 2):
                                for m in range(4):
                                    emit_wh(pz, prev, ptl, m, [k], stop_at=-1)
                            for m in (0, 1):
                                emit_wh(pz, prev, ptl, m,
                                        [KH - 2, KH - 1], stop_at=KH - 1)
                            emit_tanh(pz, 0, tl)
                            for m in (2, 3):
                                emit_wh(pz, prev, ptl, m,
                                        [KH - 2, KH - 1], stop_at=KH - 1)
                            emit_tanh(pz, 1, tl)
                        elif defer_k7:
                            # defer k6/k7 of m0/m1: pair3's fused tanh makes
                            # s[6] only as fresh as s[7] (~500ns into step t)
                            emit_wh(pz, prev, ptl, 0, range(KH - 2), stop_at=-1)
                            emit_wh(pz, prev, ptl, 1, range(KH - 2), stop_at=-1)
                            emit_wh(pz, prev, ptl, 0, [KH - 2, KH - 1], stop_at=KH - 1)
                            emit_wh(pz, prev, ptl, 1, [KH - 2, KH - 1], stop_at=KH - 1)
                            emit_tanh(pz, 0, tl)
                            emit_wh(pz, prev, ptl, 2, range(KH), stop_at=KH - 1)
                            emit_wh(pz, prev, ptl, 3, range(KH), stop_at=KH - 1)
                            emit_tanh(pz, 1, tl)
                        else:
                            emit_wh(pz, prev, ptl, 0, range(KH), stop_at=KH - 1)
                            emit_wh(pz, prev, ptl, 1, range(KH), stop_at=KH - 1)
                            emit_tanh(pz, 0, tl)
                            emit_wh(pz, prev, ptl, 2, range(KH), stop_at=KH - 1)
                            emit_wh(pz, prev, ptl, 3, range(KH), stop_at=KH - 1)
                            emit_tanh(pz, 1, tl)
                        # m6/m7 Wx mid-step: their bank frees ~650ns in
                        for m in (6, 7):
                            emit_wx(pz, xt_sb, m)
                        emit_wh(pz, prev, ptl, 4, range(KH), stop_at=KH - 1)
                        emit_wh(pz, prev, ptl, 5, range(KH), stop_at=KH - 1)
                        emit_tanh(pz, 2, tl)
                        emit_wh(pz, prev, ptl, 6, range(KH), stop_at=KH - 1)
                        emit_wh(pz, prev, ptl, 7, range(KH), stop_at=KH - 1)
                        emit_tanh(pz, 3, tl)

                    if tl == w_steps - 1:
                        pending_proj.append((t // w_steps, stage_cur))
                        if proj_pp[0] is None:
                            prep_pp()  # bootstrap before the first drain

                # tail: drain remaining windows
                for w_idx, stiles in pending_proj:
                    for ob in range(OBK):
                        for j in range(ppc):
                            emit_proj_piece(w_idx, stiles, ob, j)

            if reps > 1:
                with tc.For_i(0, reps, 1):
                    emit_whole_kernel()
            else:
                emit_whole_kernel()

    nc.compile()
    return nc


def _host_prep(x, Wx, Wh, b, Wout, bout, t_steps):
    """Build the 8 per-core input maps."""
    xt = np.ascontiguousarray(x[:, :, :t_steps].transpose(2, 1, 0)).astype(np_bf16)
    wh = Wh.astype(np_bf16)
    wx = Wx.astype(np_bf16)
    wo = Wout.astype(np_bf16)
    bbc = np.ascontiguousarray(
        np.broadcast_to(
            np.asarray(b, np.float32).reshape(KH, P).T[:, :, None], (P, KH, B)
        )
    )
    bo = np.ascontiguousarray(bout, dtype=np.float32)
    in_maps = []
    for c in range(NCORES):
        xt_c = np.ascontiguousarray(np.roll(xt, -MB * c, axis=2))
        in_maps.append(
            {
                "xt": xt_c,
                "wh": wh,
                "wx": wx,
                "wout": wo,
                "bvecbc": bbc,
                "boutvec": bo,
            }
        )
    return in_maps


def _assemble(results, t_steps, w_steps, parity=True):
    nw = t_steps // w_steps
    out = np.empty((B, t_steps, O), np.float32)
    for c in range(NCORES):
        arr = results[c]["out"].reshape(nw, OBK, P, 2, w_steps // 2, MB)
        # out[MB*c+j, w*W + tt*2 + par, ob*P + p] = arr[w, ob, p, par, tt, j]
        out[MB * c : MB * (c + 1)] = (
            arr.transpose(5, 0, 4, 3, 1, 2).reshape(MB, t_steps, O)
        )
    return out


def run(
    x, Wx, Wh, b, Wout, bout,
    t_steps=T, w_steps=64, zbufs=4, parity=True, trace=False,
):
    nc = build_program(t_steps, w_steps, zbufs=zbufs, parity=parity)
    in_maps = _host_prep(x, Wx, Wh, b, Wout, bout, t_steps)
    res = run_bass_kernel_spmd(nc, in_maps, list(range(NCORES)), trace=trace)
    out = _assemble(res.results, t_steps, w_steps, parity=parity)
    return out, res


def kernel(x, Wx, Wh, b, Wout, bout):
    out, _ = run(
        np.asarray(x, dtype=np.float32),
        np.asarray(Wx, dtype=np.float32),
        np.asarray(Wh, dtype=np.float32),
        np.asarray(b, dtype=np.float32),
        np.asarray(Wout, dtype=np.float32),
        np.asarray(bout, dtype=np.float32),
    )
    return out



# revision 8
# speedup vs baseline: 1.1530x; 1.1292x over previous
"""Trainium2 Bass kernel v4: baseline schedule skeleton, batch-sharded,
Wx hoisted to a windowed GEMM.

HW cost model (measured): every Ldweights+Matmult pair costs ~29 ns
(weight-block load stream) regardless of moving size; chain exposure of the
baseline's pair-major 4-tanh schedule measured ~370 ns/step on hw.

v4 = the baseline's proven schedule with:
  - moving 8 (own batch shard) instead of 64 (replicated),
  - the 16 in-loop Wx pairs/step replaced by a windowed u = x@Wx + b GEMM
    (16 moving-512 pairs per 64-step window, 0.25 pairs/step),
  - psum pair banks preloaded from the u window by DVE (was: bias bbc),
  - proj drain compressed to 2 pieces/step in the first half of each window;
    u GEMM pieces run in the second half, sharing the psum pool.

Per-step pair budget: 64 Wh + ~1 filler -> ~65 pairs vs baseline's 81.
"""

import numpy as np
import ml_dtypes

import concourse.bass as bass
from concourse import bacc
import concourse.mybir as mybir
import concourse.tile as tile
from concourse.bass_utils import run_bass_kernel_spmd

B, F, T = 64, 256, 1024
H, O = 1024, 512
NCORES = 8
MB = B // NCORES  # 8
P = 128
KH, KF, MH, OBK = H // P, F // P, H // P, O // P  # 8, 2, 8, 4
NPAIR = MH // 2

BF16 = mybir.dt.bfloat16
F32 = mybir.dt.float32
np_bf16 = ml_dtypes.bfloat16


def build_program(
    t_steps: int = T,
    w_steps: int = 64,
    reps: int = 1,
    defer_k7: int = 1,
    wh_k_lim: int = KH,   # TIMING DIAGNOSTIC only
) -> bass.Bass:
    assert t_steps % w_steps == 0
    nw = t_steps // w_steps
    pw = w_steps * MB        # 512
    half = pw // 2
    hw_ = w_steps // 2
    ppc = 2 * MH             # proj pieces per ob chunk (16, moving 256)

    nc = bacc.Bacc()

    xt_d = nc.declare_dram_parameter(
        "xt", [nw, P, KF, w_steps, MB], BF16, isOutput=False
    )
    wh_d = nc.declare_dram_parameter("wh", [P, KH * MH, P], BF16, isOutput=False)
    wx_d = nc.declare_dram_parameter("wx", [P, KF * MH, P], BF16, isOutput=False)
    wo_d = nc.declare_dram_parameter("wout", [P, MH * OBK, P], BF16, isOutput=False)
    bv_d = nc.declare_dram_parameter("bvec", [P, MH], F32, isOutput=False)
    bbc_d = nc.declare_dram_parameter("bvecbc", [P, MH, MB], F32, isOutput=False)
    bo_d = nc.declare_dram_parameter("boutvec", [O], F32, isOutput=False)
    out_d = nc.declare_dram_parameter("out", [nw, OBK, P, pw], F32, isOutput=True)

    with tile.TileContext(nc) as tc:
        with (
            tc.tile_pool(name="const", bufs=1) as cpool,
            tc.tile_pool(name="stage", bufs=2) as spool,
            tc.tile_pool(name="usb", bufs=2) as upool,
            tc.tile_pool(name="xin", bufs=2) as xpool,
            tc.tile_pool(name="outsb", bufs=4) as opool,
            tc.tile_pool(name="psz", bufs=1, space="PSUM") as zpool,
            tc.tile_pool(name="psp", bufs=1, space="PSUM") as ppool,
        ):
            wh_sb = cpool.tile([P, KH * MH, P], BF16, tag="wh")
            nc.sync.dma_start(wh_sb[:], wh_d[:, :, :])
            wx_sb = cpool.tile([P, KF * MH, P], BF16, tag="wx")
            nc.sync.dma_start(wx_sb[:], wx_d[:, :, :])
            wo_sb = cpool.tile([P, MH * OBK, P], BF16, tag="wo")
            nc.sync.dma_start(wo_sb[:], wo_d[:, :, :])
            bv_sb = cpool.tile([P, MH], F32, tag="bv")
            nc.sync.dma_start(bv_sb[:], bv_d[:, :])
            bbc_sb = cpool.tile([P, MH, MB], F32, tag="bbc")
            nc.sync.dma_start(bbc_sb[:], bbc_d[:, :, :])
            bo_sb = cpool.tile([P, OBK], F32, tag="bo")
            nc.sync.dma_start(bo_sb[:], bo_d.rearrange("(m p) -> p m", p=P))

            def emit_whole_kernel():
                stage_prev = [None]
                stage_cur = [None]
                u_cur = [None]
                u_next = [None]
                x_next = [None]
                pending_proj = []
                proj_pp = [None]
                ug_pp = [None]
                dve_jobs = []
                tail_mode = [False]

                def load_x(w_idx):
                    x_next[0] = xpool.tile(
                        [P, KF, w_steps, MB], BF16, tag="xw", name="xw"
                    )
                    nc.sync.dma_start(x_next[0][:], xt_d[w_idx])

                def emit_ugemm_piece(j, now_evac=False):
                    m, kf = j // 2, j % 2
                    if kf == 0:
                        ug_pp[0] = ppool.tile(
                            [P, pw], F32, tag="pp", bufs=2, name="ug"
                        )
                    nc.tensor.matmul(
                        ug_pp[0][:],
                        wx_sb[:, kf * MH + m, :],
                        x_next[0][:, kf, :, :],
                        start=(kf == 0), stop=(kf == KF - 1),
                        skip_group_check=True,
                    )
                    if kf == KF - 1:
                        pp, usb = ug_pp[0], u_next[0]

                        def evac(pp=pp, usb=usb, m=m):
                            nc.vector.tensor_scalar_add(
                                usb[:, :, m, :],
                                pp.rearrange("p (t j) -> p t j", j=MB),
                                bv_sb[:, m:m + 1],
                            )
                        if now_evac:
                            evac()
                        else:
                            dve_jobs.append(evac)

                def emit_proj_piece(w_idx, stiles, ob, j):
                    # piece j of ob's [P, pw] chunk: par = j%2, m = j//2
                    par, m = j % 2, j // 2
                    if j == 0:
                        proj_pp[0] = ppool.tile(
                            [P, pw], F32, tag="pp", bufs=2, name="pproj"
                        )
                    pp = proj_pp[0]
                    nc.tensor.matmul(
                        pp[:, par * half:(par + 1) * half],
                        wo_sb[:, m * OBK + ob, :],
                        stiles[m // 2][par][:, m % 2, :, :],
                        start=(j == 0),
                        stop=(j >= ppc - 2),
                        skip_group_check=True,
                    )
                    if j == ppc - 1:
                        def fin(pp=pp, w_idx=w_idx, ob=ob):
                            osb = opool.tile(
                                [P, pw], F32, tag="osb", name="osb"
                            )
                            nc.vector.tensor_scalar_add(
                                osb, pp, bo_sb[:, ob:ob + 1]
                            )
                            nc.sync.dma_start(out_d[w_idx, ob], osb)
                        if tail_mode[0]:
                            fin()
                        else:
                            dve_jobs.append(fin)

                def new_stage():
                    # stage[j][par]: [P, 2, hw, MB]; m-blocks 2j, 2j+1
                    return [
                        [
                            spool.tile(
                                [P, 2, hw_, MB], BF16,
                                tag=f"stage{j}p{par}", name=f"stage{j}p{par}",
                            )
                            for par in range(2)
                        ]
                        for j in range(NPAIR)
                    ]

                def state_slice(stage, k, t_local):
                    return stage[k // 2][t_local % 2][:, k % 2, t_local // 2, :]

                def pz_slot(pz, m):
                    return pz[m // 2][:, m % 2, :]

                def emit_wh(pz, prev, ptl, m, ks, stop_at):
                    for k in ks:
                        if k >= wh_k_lim:
                            continue
                        nc.tensor.matmul(
                            pz_slot(pz, m),
                            wh_sb[:, k * MH + m, :],
                            state_slice(prev, k, ptl),
                            start=False,
                            stop=(k == min(stop_at, wh_k_lim - 1)),
                            skip_group_check=True,
                        )

                def emit_tanh(pz, j, tl):
                    nc.scalar.activation(
                        stage_cur[0][j][tl % 2][:, :, tl // 2, :],
                        pz[j],
                        mybir.ActivationFunctionType.Tanh,
                    )

                def preload(pz, j, t):
                    # pz[j] <- u[t] (or bias const in window 0, which runs
                    # the in-loop Wx path) for m-blocks 2j, 2j+1
                    w_rel = t // w_steps
                    if w_rel == 0:
                        nc.vector.tensor_copy(
                            pz[j], bbc_sb[:, 2 * j:2 * j + 2, :]
                        )
                        return
                    usb = u_next[0] if w_rel > cur_w[0] else u_cur[0]
                    nc.vector.tensor_copy(
                        pz[j], usb[:, t % w_steps, 2 * j:2 * j + 2, :]
                    )

                cur_w = [0]

                # ---- prologue: window 0 runs the SAFE in-loop Wx path
                # (the u-GEMM burst here raced on hw); u windows start at w=1
                load_x(0)
                x0 = x_next[0]
                u_cur[0] = None

                pz_next = [None]  # psum tiles for step t+1, preloaded early

                def alloc_pz():
                    return [
                        zpool.tile(
                            [P, 2, MB], F32,
                            tag=f"pair{j}", bufs=(2 if j < 2 else 1),
                            name=f"pair{j}",
                        )
                        for j in range(NPAIR)
                    ]

                for t in range(t_steps):
                    tl = t % w_steps
                    w_idx = t // w_steps
                    cur_w[0] = w_idx
                    if tl == 0:
                        stage_prev[0] = stage_cur[0]
                        stage_cur[0] = new_stage()
                        if w_idx + 1 < nw:
                            u_next[0] = upool.tile(
                                [P, w_steps, MH, MB], F32, tag="u", name="u"
                            )
                            load_x(w_idx + 1)

                    # psum tiles for this step: pairs 0-2 were preloaded
                    # during step t-1 (double-buffered); pair 3 is single-
                    # buffered, preload it now (its tanh just ran).
                    if pz_next[0] is None:
                        pz = alloc_pz()
                        for j in range(NPAIR):
                            preload(pz, j, t)
                    else:
                        pz = pz_next[0]
                        preload(pz, 2, t)
                        preload(pz, 3, t)
                    # preload pairs 0..2 of step t+1 (banks freed by their
                    # step-t tanhs as the step progresses; DVE parks briefly)
                    if t + 1 < t_steps:
                        pz_next[0] = alloc_pz()

                    # deferred DVE work: pop EVERY step (1-step deferral
                    # avoids DVE-SEQ head-parking on unready waits, while
                    # staying far inside the psum pool's bank reuse
                    # distance -- a reader emitted after its bank is
                    # re-allocated would be an untracked race)
                    if dve_jobs:
                        dve_jobs.pop(0)()

                    # ---- fillers: proj 2/step at tl 0..31, u at 32..62 ---
                    filler_jobs = []
                    if pending_proj and tl < 32:
                        w_i, stiles = pending_proj[0]
                        ob, j0 = tl // 8, (tl % 8) * 2
                        filler_jobs.append(
                            lambda w_i=w_i, stiles=stiles, ob=ob, j0=j0: (
                                emit_proj_piece(w_i, stiles, ob, j0),
                                emit_proj_piece(w_i, stiles, ob, j0 + 1),
                            )
                        )
                        if tl == 31:
                            pending_proj.pop(0)
                    if w_idx + 1 < nw and tl >= 32 and tl % 2 == 0:
                        jx = (tl - 32) // 2
                        filler_jobs.append(
                            lambda jx=jx: emit_ugemm_piece(jx)
                        )

                    def emit_fillers():
                        for fj in filler_jobs:
                            fj()
                        filler_jobs.clear()

                    def emit_wx_inloop():
                        if w_idx != 0:
                            return
                        for m in range(MH):
                            for kf in range(KF):
                                nc.tensor.matmul(
                                    pz_slot(pz, m),
                                    wx_sb[:, kf * MH + m, :],
                                    x0[:, kf, tl, :],
                                    start=False,
                                    stop=(t == 0 and kf == KF - 1),
                                    skip_group_check=True,
                                )

                    # ---- the step ----------------------------------------
                    if t == 0:
                        emit_fillers()
                        emit_wx_inloop()
                        for j in range(NPAIR):
                            emit_tanh(pz, j, tl)
                    else:
                        prev = stage_cur[0] if tl > 0 else stage_prev[0]
                        ptl = (t - 1) % w_steps
                        emit_fillers()
                        emit_wx_inloop()
                        if defer_k7:
                            emit_wh(pz, prev, ptl, 0, range(KH - 2), KH - 1)
                            emit_wh(pz, prev, ptl, 1, range(KH - 2), KH - 1)
                            emit_wh(pz, prev, ptl, 0, [KH - 2, KH - 1], KH - 1)
                            emit_wh(pz, prev, ptl, 1, [KH - 2, KH - 1], KH - 1)
                            emit_tanh(pz, 0, tl)
                            emit_wh(pz, prev, ptl, 2, range(KH), KH - 1)
                            emit_wh(pz, prev, ptl, 3, range(KH), KH - 1)
                            emit_tanh(pz, 1, tl)
                        else:
                            emit_wh(pz, prev, ptl, 0, range(KH), KH - 1)
                            emit_wh(pz, prev, ptl, 1, range(KH), KH - 1)
                            emit_tanh(pz, 0, tl)
                            emit_wh(pz, prev, ptl, 2, range(KH), KH - 1)
                            emit_wh(pz, prev, ptl, 3, range(KH), KH - 1)
                            emit_tanh(pz, 1, tl)
                        emit_wh(pz, prev, ptl, 4, range(KH), KH - 1)
                        emit_wh(pz, prev, ptl, 5, range(KH), KH - 1)
                        emit_tanh(pz, 2, tl)
                        emit_wh(pz, prev, ptl, 6, range(KH), KH - 1)
                        emit_wh(pz, prev, ptl, 7, range(KH), KH - 1)
                        emit_tanh(pz, 3, tl)

                    # preload pairs 0..1 of t+1 after this step's tanhs
                    if t + 1 < t_steps:
                        for j in range(2):
                            preload(pz_next[0], j, t + 1)

                    if tl == w_steps - 1:
                        pending_proj.append((w_idx, stage_cur[0]))
                        if w_idx + 1 < nw:
                            u_cur[0] = u_next[0]

                # tail
                for job in dve_jobs:
                    job()
                dve_jobs.clear()
                tail_mode[0] = True
                for w_i, stiles in pending_proj:
                    for ob in range(OBK):
                        for j in range(ppc):
                            emit_proj_piece(w_i, stiles, ob, j)

            if reps > 1:
                with tc.For_i(0, reps, 1):
                    emit_whole_kernel()
            else:
                emit_whole_kernel()

    nc.compile()
    return nc


def _host_prep(x, Wx, Wh, b, Wout, bout, t_steps, w_steps=64):
    nw = t_steps // w_steps
    xt = np.ascontiguousarray(
        x[:, :, :t_steps].transpose(2, 1, 0)
    ).astype(np_bf16)  # [T, F, B]
    wh = np.ascontiguousarray(
        np.asarray(Wh).reshape(KH, P, MH, P).transpose(1, 0, 2, 3)
        .reshape(P, KH * MH, P)
    ).astype(np_bf16)
    wx = np.ascontiguousarray(
        np.asarray(Wx).reshape(KF, P, MH, P).transpose(1, 0, 2, 3)
        .reshape(P, KF * MH, P)
    ).astype(np_bf16)
    wo = np.ascontiguousarray(
        np.asarray(Wout).reshape(MH, P, OBK, P).transpose(1, 0, 2, 3)
        .reshape(P, MH * OBK, P)
    ).astype(np_bf16)
    bv = np.ascontiguousarray(np.asarray(b, np.float32).reshape(MH, P).T)
    bbc = np.ascontiguousarray(
        np.broadcast_to(bv[:, :, None], (P, MH, MB))
    )
    bo = np.ascontiguousarray(bout, dtype=np.float32)
    in_maps = []
    for c in range(NCORES):
        xc = xt[:, :, MB * c:MB * (c + 1)]
        xw = np.ascontiguousarray(
            xc.reshape(nw, w_steps, KF, P, MB).transpose(0, 3, 2, 1, 4)
        )
        in_maps.append(
            {"xt": xw, "wh": wh, "wx": wx, "wout": wo,
             "bvec": bv, "bvecbc": bbc, "boutvec": bo}
        )
    return in_maps


def _assemble(results, t_steps, w_steps):
    nw = t_steps // w_steps
    out = np.empty((B, t_steps, O), np.float32)
    for c in range(NCORES):
        arr = results[c]["out"].reshape(nw, OBK, P, 2, w_steps // 2, MB)
        # out[MB*c+j, w*W + tt*2 + par, ob*P + p] = arr[w, ob, p, par, tt, j]
        out[MB * c:MB * (c + 1)] = (
            arr.transpose(5, 0, 4, 3, 1, 2).reshape(MB, t_steps, O)
        )
    return out


def run(x, Wx, Wh, b, Wout, bout, t_steps=T, w_steps=64, trace=False):
    nc = build_program(t_steps, w_steps)
    in_maps = _host_prep(x, Wx, Wh, b, Wout, bout, t_steps, w_steps)
    res = run_bass_kernel_spmd(nc, in_maps, list(range(NCORES)), trace=trace)
    out = _assemble(res.results, t_steps, w_steps)
    return out, res


def kernel(x, Wx, Wh, b, Wout, bout):
    out, _ = run(
        np.asarray(x, dtype=np.float32),
        np.asarray(Wx, dtype=np.float32),
        np.asarray(Wh, dtype=np.float32),
        np.asarray(b, dtype=np.float32),
        np.asarray(Wout, dtype=np.float32),
        np.asarray(bout, dtype=np.float32),
    )
    return out
